# revision 7
# baseline (speedup 1.0000x reference)
"""Causal attention (B=8, S=2048, D=768, single head) on 8 trn2 NeuronCores.

Sharding: data-parallel over batch — core b computes batch element b.

All matmul operands are bf16 (f32 accumulation in PSUM); ~5e-3 rel err,
well inside the 2e-2 gate.

Algebraic trick: scores = Q K^T = x (Wq Wk^T) x^T. M = Wq @ Wk^T is
computed at startup from the weights alone, so EVERY chunk needs just
one projection B = (x M)^T and contracts scores^T = XT.T @ B — the
separate Q and K projections disappear entirely.

x^T production is offloaded from the PE where possible: chunk 0 is
cast-loaded to bf16 by the SWDGE (gpsimd) ring and PE-transposed during
the startup weight-DMA window (which would otherwise idle the PE);
chunks 1-3 are cast to a bf16 DRAM scratch (SWDGE, DRAM->DRAM) and
xbar-DMA-transposed straight into XT slices on the sync HWDGE ring.

Startup: wq/wk f32 pieces stream on the two HWDGE rings and are
f32r-PE-transposed as they arrive (filling the DMA-bound window,
interleaved with x(0) transposes and a few HAM-warming dummy matmuls);
M's 256-wide matmul groups run as wk pieces complete; wv is cast-loaded
directly into its bf16 SBUF tile by the SWDGE ring.

Per-core pipeline (fused over 512-wide s-chunks):
  1. B = M-block.T @ xT per chunk (chunk 0 also V(0) first, to fill the
     M-wait)
  2. scores^T tile [k_part, q]: contract d via XT, B
  3. exp (scale fused) on ACT -> PT bf16; triangular mask on corner block
  4. out[q, e] (+denominator via ones cols of V) = PT_blk.T @ V_blk
  5. normalize via reciprocal (DVE) + scaled copy (ACT), stores on sync
"""

import numpy as np

import concourse.bass as bass
import concourse.mybir as mybir
from concourse import bacc
from concourse.tile import TileContext
from concourse.bass_utils import run_bass_kernel_spmd
from concourse.masks import make_identity

B, S, D = 8, 2048, 768
P = 128
ND = D // P            # 6 feature blocks
NB = S // P            # 16 seq blocks
CH = 512               # s-chunk width
NCH = S // CH          # 4 chunks
QPC = CH // P          # 4 q/s-blocks per chunk
SCALE = 1.0 / float(np.sqrt(D))
F32 = mybir.dt.float32
F32R = mybir.dt.float32r
BF16 = mybir.dt.bfloat16
EXP = mybir.ActivationFunctionType.Exp


def _build_nc():
    nc = bacc.Bacc(None, target_bir_lowering=False)
    xb = nc.dram_tensor("xb", [S, D], F32, kind="ExternalInput")
    wq_d = nc.dram_tensor("wq", [D, D], F32, kind="ExternalInput")
    wk_d = nc.dram_tensor("wk", [D, D], F32, kind="ExternalInput")
    wv_d = nc.dram_tensor("wv", [D, D], F32, kind="ExternalInput")
    out_d = nc.dram_tensor("out", [S, D], F32, kind="ExternalOutput")
    # bf16 x scratch in transpose-friendly layout: [d_block, s, d_in] so
    # each xbar-transpose source [512, 128] is fully contiguous (the xbar
    # ucode path corrupts even partitions with wide strided source rows)
    xbf_d = nc.dram_tensor("xbf", [ND, S, P], BF16, kind="Internal")

    # [d, e] weight views as [d_in(128), d_block(6), e(768)]
    wq_r = wq_d[:, :].rearrange("(o p) e -> p o e", p=P)
    wk_r = wk_d[:, :].rearrange("(o p) e -> p o e", p=P)
    wv_r = wv_d[:, :].rearrange("(o p) e -> p o e", p=P)

    with TileContext(nc) as tc:
        with (
            tc.tile_pool(name="const", bufs=1) as constp,
            tc.tile_pool(name="persist", bufs=1) as persist,
            tc.tile_pool(name="wstage", bufs=4) as wstage,
            tc.tile_pool(name="xload", bufs=4) as xload,
            tc.tile_pool(name="qt", bufs=2) as qtp,
            tc.tile_pool(name="outp", bufs=2) as outp,
            tc.tile_pool(name="rc", bufs=4) as rcp,
            tc.tile_pool(name="psW", bufs=4, space="PSUM") as psW,
            tc.tile_pool(name="psO", bufs=4, space="PSUM") as psO,
        ):
            ident = constp.tile([P, P], F32)
            make_identity(nc, ident)
            ident_r = constp.tile([P, P], F32R)
            nc.vector.tensor_copy(ident_r, ident)
            ident_b = constp.tile([P, P], BF16)
            nc.vector.tensor_copy(ident_b, ident)
            # tri[p, j] = 1.0 if p <= j else 0.0 (keep k <= q on the
            # diagonal 128x128 corner of each score block)
            tri = constp.tile([P, P], BF16)
            nc.gpsimd.memset(tri, 1.0)
            nc.gpsimd.affine_select(
                out=tri,
                in_=tri,
                compare_op=mybir.AluOpType.is_ge,
                fill=0.0,
                base=0,
                pattern=[[1, P]],
                channel_multiplier=-1,
            )

            WqT = persist.tile([P, ND, D], BF16)     # Wq^T: [e_in, eo, d]
            WkT = persist.tile([P, ND, D], BF16)     # Wk^T: [e_in, eo, d]
            Mt = persist.tile([P, ND, D], BF16)      # M:    [d'_in, o', d]
            WV = persist.tile([P, ND, D], BF16)
            XT = persist.tile([P, ND, S], BF16)      # x^T, all chunks
            V = persist.tile([P, NB, D + 2], BF16)   # [s_in, sb, e]; cols D..D+1 = 1.0
            PT = persist.tile([P, NB, CH], BF16)     # exp(scores^T) blocks of chunk
            ones_col = constp.tile([P, NB, 2], BF16)
            nc.vector.memset(ones_col, 1.0)
            nc.vector.tensor_copy(V[:, :, D : D + 2], ones_col)

            # ---- startup DMAs. wq/wk f32 pieces split across the two
            # HWDGE rings (sync/scalar); x chunk 0 cast-loads to bf16 on
            # the gpsimd SWDGE ring, followed by wv (cast straight into
            # WV) and the DRAM->DRAM bf16 casts of x chunks 1-3.
            wq_stage = []
            wk_stage = []
            for o in range(ND):
                s = wstage.tile([P, 1, D], F32R, tag="ws", name=f"wq{o}")
                eng = nc.sync if o % 2 == 0 else nc.scalar
                eng.dma_start(s, wq_r[:, o : o + 1, :].bitcast(F32R))
                wq_stage.append(s)
            for o in range(ND):
                s = wstage.tile([P, 1, D], F32R, tag="ws", name=f"wk{o}")
                eng = nc.sync if o % 2 == 0 else nc.scalar
                eng.dma_start(s, wk_r[:, o : o + 1, :].bitcast(F32R))
                wk_stage.append(s)
            xf_tiles = []
            for sb in range(QPC):
                xf = xload.tile([P, D], BF16, tag="xf", name=f"xf{sb}")
                nc.gpsimd.dma_start(xf, xb[sb * P : (sb + 1) * P, :])
                xf_tiles.append(xf)
            nc.gpsimd.dma_start(WV, wv_r)
            for c in range(1, NCH):
                nc.gpsimd.dma_start(
                    xbf_d[:, c * CH : (c + 1) * CH, :],
                    xb[c * CH : (c + 1) * CH, :].rearrange("s (o p) -> o s p", p=P),
                )

            # XT for chunks 1-3 via xbar DMA-transpose on the sync ring
            # (sync has no compute duties; its only later work is the out
            # stores, emitted well after these clear).
            for c in range(1, NCH):
                for o in range(ND):
                    nc.sync.dma_start(
                        XT[:, o, c * CH : (c + 1) * CH],
                        xbf_d[o, c * CH : (c + 1) * CH, :],
                        transpose=True,
                    )

            # ---- PE startup: transpose weight/x pieces as they arrive,
            # with dummy matmuls sprinkled in to warm the HAM clock gate
            # (transpose-mode does not count as PE-busy for HAM).
            def emit_warm(n):
                for _ in range(n):
                    w = psO.tile([P, P], F32, tag="o", name="warm")
                    nc.tensor.matmul(w, ident_b, ident_b, start=True, stop=True)

            def emit_wtransS(stage, WT, o):
                # f32r-transpose one staged f32 weight piece [d-block o, e]
                # into WT[:, :, o-block]; the PSUM->SBUF copies cast bf16.
                for h in range(2):
                    ps_w = psW.tile([P, 3 * P], F32R, tag="w")
                    for eh in range(3):
                        eo = h * 3 + eh
                        nc.tensor.transpose(
                            ps_w[:, eh * P : (eh + 1) * P],
                            stage[:, 0, eo * P : (eo + 1) * P],
                            ident_r,
                        )
                    nc.vector.tensor_copy(
                        WT[:, h * 3 : h * 3 + 3, o * P : (o + 1) * P],
                        ps_w.bitcast(F32).rearrange("p (o s) -> p o s", o=3),
                    )

            def emit_xtrans0(sb):
                # bf16 PE transpose of a cast-loaded x(0) tile into XT
                xr = xf_tiles[sb]
                for h in range(2):
                    ps_t = psW.tile([P, 3 * P], BF16, tag="w")
                    for dh in range(3):
                        do = h * 3 + dh
                        nc.tensor.transpose(
                            ps_t[:, dh * P : (dh + 1) * P],
                            xr[:, do * P : (do + 1) * P],
                            ident_b,
                        )
                    nc.vector.tensor_copy(
                        XT[:, h * 3 : h * 3 + 3, sb * P : (sb + 1) * P],
                        ps_t.rearrange("p (o s) -> p o s", o=3),
                    )

            def emit_m_group(b2):
                # M column pair-group: Mt[:, :, b2*256:(b2+1)*256]
                for a in range(ND):
                    pm = psW.tile([P, 2 * P], F32, tag="w")
                    for eo in range(ND):
                        nc.tensor.matmul(
                            pm,
                            WqT[:, eo, a * P : (a + 1) * P],
                            WkT[:, eo, b2 * 2 * P : (b2 + 1) * 2 * P],
                            start=(eo == 0),
                            stop=(eo == ND - 1),
                        )
                    nc.vector.tensor_copy(
                        Mt[:, a, b2 * 2 * P : (b2 + 1) * 2 * P], pm
                    )

            emit_wtransS(wq_stage[0], WqT, 0)
            emit_warm(8)
            emit_wtransS(wq_stage[1], WqT, 1)
            emit_xtrans0(0)
            emit_warm(4)
            emit_wtransS(wq_stage[2], WqT, 2)
            emit_xtrans0(1)
            emit_warm(4)
            emit_wtransS(wq_stage[3], WqT, 3)
            emit_xtrans0(2)
            emit_warm(4)
            emit_wtransS(wq_stage[4], WqT, 4)
            emit_xtrans0(3)
            emit_wtransS(wq_stage[5], WqT, 5)
            emit_warm(4)
            emit_wtransS(wk_stage[0], WkT, 0)
            emit_wtransS(wk_stage[1], WkT, 1)
            emit_m_group(0)
            emit_wtransS(wk_stage[2], WkT, 2)
            emit_wtransS(wk_stage[3], WkT, 3)
            emit_m_group(1)
            emit_wtransS(wk_stage[4], WkT, 4)
            emit_wtransS(wk_stage[5], WkT, 5)
            emit_m_group(2)

            # ---- V projection for chunk 0 up front: its inputs (XT(0),
            # WV) are ready before Mt, so it fills the tail of the
            # startup window while the last M groups finish.
            def emit_vproj(c):
                for sb in range(QPC):
                    xt_blk = XT[:, :, (c * QPC + sb) * P : (c * QPC + sb + 1) * P]
                    pv0 = psW.tile([P, CH], F32, tag="w")
                    for do in range(ND):
                        nc.tensor.matmul(
                            pv0,
                            xt_blk[:, do, :],
                            WV[:, do, 0:CH],
                            start=(do == 0),
                            stop=(do == ND - 1),
                        )
                    nc.scalar.copy(V[:, c * QPC + sb, 0:CH], pv0)
                    pv1 = psW.tile([P, CH], F32, tag="w")
                    for do in range(ND):
                        nc.tensor.matmul(
                            pv1[:, 0 : D - CH],
                            xt_blk[:, do, :],
                            WV[:, do, CH:D],
                            start=(do == 0),
                            stop=(do == ND - 1),
                        )
                    nc.scalar.copy(V[:, c * QPC + sb, CH:D], pv1[:, 0 : D - CH])

            emit_vproj(0)

            for c in range(NCH):
                # ---- projection: B = (x M)^T = Mt.T-contracted with XT
                QT = qtp.tile([P, ND, CH], BF16, tag="qt")
                for eb in range(ND):
                    pq = psW.tile([P, CH], F32, tag="w")
                    for do in range(ND):
                        nc.tensor.matmul(
                            pq,
                            Mt[:, do, eb * P : (eb + 1) * P],
                            XT[:, do, c * CH : (c + 1) * CH],
                            start=(do == 0),
                            stop=(do == ND - 1),
                        )
                    nc.vector.tensor_copy(QT[:, eb, :], pq)

                # ---- scores^T + exp; triangular mask on the diagonal corner
                for kb in range(QPC * (c + 1)):
                    i = kb - QPC * c
                    q0 = max(i, 0) * P
                    W = CH - q0
                    ps_s = psW.tile([P, CH], F32, tag="w")
                    for eo in range(ND):
                        nc.tensor.matmul(
                            ps_s[:, 0:W],
                            XT[:, eo, kb * P : (kb + 1) * P],
                            QT[:, eo, q0:CH],
                            start=(eo == 0),
                            stop=(eo == ND - 1),
                        )
                    nc.scalar.activation(PT[:, kb, q0:CH], ps_s[:, 0:W], EXP, scale=SCALE)
                    if i >= 0:
                        nc.vector.tensor_mul(
                            PT[:, kb, q0 : q0 + P], PT[:, kb, q0 : q0 + P], tri
                        )

                # ---- V projection for this chunk (chunk 0 ran up front)
                if c > 0:
                    emit_vproj(c)

                # ---- attn @ [V | 1], normalize, store
                for qs in range(QPC):
                    qb = c * QPC + qs
                    po0 = psO.tile([P, CH], F32, tag="o")
                    po1 = psW.tile([P, D + 2 - CH], F32, tag="w")
                    for kb in range(qb + 1):
                        nc.tensor.matmul(
                            po1,
                            PT[:, kb, qs * P : (qs + 1) * P],
                            V[:, kb, CH : D + 2],
                            start=(kb == 0),
                            stop=(kb == qb),
                        )
                    recip = rcp.tile([P, 1], F32, tag="rc")
                    nc.vector.reciprocal(recip, po1[:, D - CH : D - CH + 1])
                    o_sb = outp.tile([P, D], F32, tag="o")
                    nc.scalar.mul(o_sb[:, CH:D], po1[:, 0 : D - CH], recip)
                    nc.sync.dma_start(out_d[qb * P : (qb + 1) * P, CH:D], o_sb[:, CH:D])
                    for kb in range(qb + 1):
                        nc.tensor.matmul(
                            po0,
                            PT[:, kb, qs * P : (qs + 1) * P],
                            V[:, kb, 0:CH],
                            start=(kb == 0),
                            stop=(kb == qb),
                        )
                    nc.scalar.mul(o_sb[:, 0:CH], po0, recip)
                    nc.sync.dma_start(out_d[qb * P : (qb + 1) * P, 0:CH], o_sb[:, 0:CH])

    nc.finalize()
    return nc


_NC_CACHE = None


def _get_nc():
    global _NC_CACHE
    if _NC_CACHE is None:
        _NC_CACHE = _build_nc()
    return _NC_CACHE


def run(inputs, trace=False):
    x = np.asarray(inputs["x"], dtype=np.float32)
    wq = np.asarray(inputs["wq"], dtype=np.float32)
    wk = np.asarray(inputs["wk"], dtype=np.float32)
    wv = np.asarray(inputs["wv"], dtype=np.float32)
    nc = _get_nc()
    in_maps = [
        {"xb": np.ascontiguousarray(x[b]), "wq": wq, "wk": wk, "wv": wv}
        for b in range(B)
    ]
    res = run_bass_kernel_spmd(nc, in_maps, core_ids=list(range(B)), trace=trace)
    out = np.stack([r["out"] for r in res.results]).astype(np.float32)
    return out, res


def kernel(x, wq, wk, wv):
    out, _ = run({"x": x, "wq": wq, "wk": wk, "wv": wv}, trace=False)
    return out


# revision 10
# speedup vs baseline: 1.1100x; 1.1100x over previous
"""Causal attention (B=8, S=2048, D=768, single head) on 8 trn2 NeuronCores.

Sharding: data-parallel over batch — core b computes batch element b.

All matmul operands are bf16 (f32 accumulation in PSUM); ~5e-3 rel err,
well inside the 2e-2 gate.

Algebraic trick: scores = Q K^T = x (Wq Wk^T) x^T. M = Wq @ Wk^T is
computed at startup from the weights alone, so EVERY chunk needs just
one projection B = (x M)^T and contracts scores^T = XT.T @ B — the
separate Q and K projections disappear entirely.

x^T production is offloaded from the PE where possible: chunk 0 is
cast-loaded to bf16 by the SWDGE (gpsimd) ring and PE-transposed during
the startup weight-DMA window (which would otherwise idle the PE);
chunks 1-3 are cast to a bf16 DRAM scratch (SWDGE, DRAM->DRAM) and
xbar-DMA-transposed straight into XT slices on the sync HWDGE ring.

Startup: wq/wk f32 pieces stream on the two HWDGE rings and are
f32r-PE-transposed as they arrive (filling the DMA-bound window,
interleaved with x(0) transposes and a few HAM-warming dummy matmuls);
M's 256-wide matmul groups run as wk pieces complete; wv is cast-loaded
directly into its bf16 SBUF tile by the SWDGE ring.

Per-core pipeline (fused over 512-wide s-chunks):
  1. B = M-block.T @ xT per chunk (chunk 0 also V(0) first, to fill the
     M-wait)
  2. scores^T tile [k_part, q]: contract d via XT, B
  3. exp (scale fused) on ACT -> PT bf16; triangular mask on corner block
  4. out[q, e] (+denominator via ones cols of V) = PT_blk.T @ V_blk
  5. normalize via reciprocal (DVE) + scaled copy (ACT), stores on sync
"""

import numpy as np

import concourse.bass as bass
import concourse.mybir as mybir
from concourse import bacc
from concourse.tile import TileContext
from concourse.bass_utils import run_bass_kernel_spmd
from concourse.masks import make_identity

B, S, D = 8, 2048, 768
P = 128
ND = D // P            # 6 feature blocks
NB = S // P            # 16 seq blocks
CH = 512               # s-chunk width
NCH = S // CH          # 4 chunks
QPC = CH // P          # 4 q/s-blocks per chunk
SCALE = 1.0 / float(np.sqrt(D))
F32 = mybir.dt.float32
F32R = mybir.dt.float32r
BF16 = mybir.dt.bfloat16
EXP = mybir.ActivationFunctionType.Exp


def _build_nc():
    nc = bacc.Bacc(None, target_bir_lowering=False)
    xb = nc.dram_tensor("xb", [S, D], F32, kind="ExternalInput")
    wq_d = nc.dram_tensor("wq", [D, D], F32, kind="ExternalInput")
    wk_d = nc.dram_tensor("wk", [D, D], F32, kind="ExternalInput")
    wv_d = nc.dram_tensor("wv", [D, D], F32, kind="ExternalInput")
    out_d = nc.dram_tensor("out", [S, D], F32, kind="ExternalOutput")
    # bf16 x scratch in transpose-friendly layout: [d_block, s, d_in] so
    # each xbar-transpose source [512, 128] is fully contiguous (the xbar
    # ucode path corrupts even partitions with wide strided source rows)
    xbf_d = nc.dram_tensor("xbf", [ND, S, P], BF16, kind="Internal")

    # [d, e] weight views as [d_in(128), d_block(6), e(768)]
    wq_r = wq_d[:, :].rearrange("(o p) e -> p o e", p=P)
    wk_r = wk_d[:, :].rearrange("(o p) e -> p o e", p=P)
    wv_r = wv_d[:, :].rearrange("(o p) e -> p o e", p=P)

    with TileContext(nc) as tc:
        with (
            tc.tile_pool(name="const", bufs=1) as constp,
            tc.tile_pool(name="persist", bufs=1) as persist,
            tc.tile_pool(name="wstage", bufs=4) as wstage,
            tc.tile_pool(name="xload", bufs=4) as xload,
            tc.tile_pool(name="qt", bufs=2) as qtp,
            tc.tile_pool(name="outp", bufs=2) as outp,
            tc.tile_pool(name="rc", bufs=4) as rcp,
            tc.tile_pool(name="psW", bufs=4, space="PSUM") as psW,
            tc.tile_pool(name="psO", bufs=4, space="PSUM") as psO,
        ):
            ident = constp.tile([P, P], F32)
            make_identity(nc, ident)
            ident_r = constp.tile([P, P], F32R)
            nc.vector.tensor_copy(ident_r, ident)
            ident_b = constp.tile([P, P], BF16)
            nc.vector.tensor_copy(ident_b, ident)
            # tri[p, j] = 1.0 if p <= j else 0.0 (keep k <= q on the
            # diagonal 128x128 corner of each score block)
            tri = constp.tile([P, P], BF16)
            nc.gpsimd.memset(tri, 1.0)
            nc.gpsimd.affine_select(
                out=tri,
                in_=tri,
                compare_op=mybir.AluOpType.is_ge,
                fill=0.0,
                base=0,
                pattern=[[1, P]],
                channel_multiplier=-1,
            )

            WqT = persist.tile([P, ND, D], BF16)     # Wq^T: [e_in, eo, d]
            WkT = persist.tile([P, ND, D], BF16)     # Wk^T: [e_in, eo, d]
            Mt = persist.tile([P, ND, D], BF16)      # M:    [d'_in, o', d]
            WV = persist.tile([P, ND, D], BF16)
            XT = persist.tile([P, ND, S], BF16)      # x^T, all chunks
            V = persist.tile([P, NB, D + 2], BF16)   # [s_in, sb, e]; cols D..D+1 = 1.0
            PT = persist.tile([P, NB, CH], BF16)     # exp(scores^T) blocks of chunk
            ones_col = constp.tile([P, NB, 2], BF16)
            nc.vector.memset(ones_col, 1.0)
            nc.vector.tensor_copy(V[:, :, D : D + 2], ones_col)

            # ---- startup DMAs. wq/wk f32 pieces split across the two
            # HWDGE rings (sync/scalar); x chunk 0 cast-loads to bf16 on
            # the gpsimd SWDGE ring, followed by wv (cast straight into
            # WV) and the DRAM->DRAM bf16 casts of x chunks 1-3.
            wq_stage = []
            wk_stage = []
            for o in range(ND):
                s = wstage.tile([P, 1, D], F32R, tag="ws", name=f"wq{o}")
                eng = nc.sync if o % 2 == 0 else nc.scalar
                eng.dma_start(s, wq_r[:, o : o + 1, :].bitcast(F32R))
                wq_stage.append(s)
            for o in range(ND):
                s = wstage.tile([P, 1, D], F32R, tag="ws", name=f"wk{o}")
                eng = nc.sync if o % 2 == 0 else nc.scalar
                eng.dma_start(s, wk_r[:, o : o + 1, :].bitcast(F32R))
                wk_stage.append(s)
            xf_tiles = []
            for sb in range(QPC):
                xf = xload.tile([P, D], BF16, tag="xf", name=f"xf{sb}")
                nc.gpsimd.dma_start(xf, xb[sb * P : (sb + 1) * P, :])
                xf_tiles.append(xf)
            nc.gpsimd.dma_start(WV, wv_r)
            # chunks 1-3: SWDGE cast-load to SBUF bf16 (3KB rows, 16-engine
            # swizzle — a direct DRAM->DRAM scatter cast runs on a single
            # SDMA engine at ~27GB/s, far too slow)
            xc_tiles = {}
            for c in range(1, NCH):
                xc = xload.tile([P, QPC, D], BF16, tag="xc", name=f"xc{c}")
                nc.gpsimd.dma_start(
                    xc, xb[c * CH : (c + 1) * CH, :].rearrange("(s p) d -> p s d", p=P)
                )
                xc_tiles[c] = xc

            def emit_xt_dma(c):
                # scatter-store chunk c to the o-major DRAM scratch, then
                # xbar-transpose each contiguous [512, 128] o-slice into XT.
                # All on the sync ring, in per-chunk batches interleaved
                # ahead of each AV store group (no head-of-line blocking).
                xc = xc_tiles[c]
                for o in range(ND):
                    nc.sync.dma_start(
                        xbf_d[o, c * CH : (c + 1) * CH, :].rearrange(
                            "(s p) q -> p s q", p=P
                        ),
                        xc[:, :, o * P : (o + 1) * P],
                    )
                for o in range(ND):
                    nc.sync.dma_start(
                        XT[:, o, c * CH : (c + 1) * CH],
                        xbf_d[o, c * CH : (c + 1) * CH, :],
                        transpose=True,
                    )

            # ---- PE startup: transpose weight/x pieces as they arrive,
            # with dummy matmuls sprinkled in to warm the HAM clock gate
            # (transpose-mode does not count as PE-busy for HAM).
            def emit_warm(n):
                for _ in range(n):
                    w = psO.tile([P, P], F32, tag="o", name="warm")
                    nc.tensor.matmul(w, ident_b, ident_b, start=True, stop=True)

            def emit_wtransS(stage, WT, o):
                # f32r-transpose one staged f32 weight piece [d-block o, e]
                # into WT[:, :, o-block]; the PSUM->SBUF copies cast bf16.
                for h in range(2):
                    ps_w = psW.tile([P, 3 * P], F32R, tag="w")
                    for eh in range(3):
                        eo = h * 3 + eh
                        nc.tensor.transpose(
                            ps_w[:, eh * P : (eh + 1) * P],
                            stage[:, 0, eo * P : (eo + 1) * P],
                            ident_r,
                        )
                    nc.vector.tensor_copy(
                        WT[:, h * 3 : h * 3 + 3, o * P : (o + 1) * P],
                        ps_w.bitcast(F32).rearrange("p (o s) -> p o s", o=3),
                    )

            def emit_xtrans0(sb):
                # bf16 PE transpose of a cast-loaded x(0) tile into XT
                xr = xf_tiles[sb]
                for h in range(2):
                    ps_t = psW.tile([P, 3 * P], BF16, tag="w")
                    for dh in range(3):
                        do = h * 3 + dh
                        nc.tensor.transpose(
                            ps_t[:, dh * P : (dh + 1) * P],
                            xr[:, do * P : (do + 1) * P],
                            ident_b,
                        )
                    nc.vector.tensor_copy(
                        XT[:, h * 3 : h * 3 + 3, sb * P : (sb + 1) * P],
                        ps_t.rearrange("p (o s) -> p o s", o=3),
                    )

            def emit_m_group(b2):
                # M column pair-group: Mt[:, :, b2*256:(b2+1)*256]
                for a in range(ND):
                    pm = psW.tile([P, 2 * P], F32, tag="w")
                    for eo in range(ND):
                        nc.tensor.matmul(
                            pm,
                            WqT[:, eo, a * P : (a + 1) * P],
                            WkT[:, eo, b2 * 2 * P : (b2 + 1) * 2 * P],
                            start=(eo == 0),
                            stop=(eo == ND - 1),
                        )
                    nc.vector.tensor_copy(
                        Mt[:, a, b2 * 2 * P : (b2 + 1) * 2 * P], pm
                    )

            emit_wtransS(wq_stage[0], WqT, 0)
            emit_warm(8)
            emit_wtransS(wq_stage[1], WqT, 1)
            emit_xtrans0(0)
            emit_warm(4)
            emit_wtransS(wq_stage[2], WqT, 2)
            emit_xtrans0(1)
            emit_warm(4)
            emit_wtransS(wq_stage[3], WqT, 3)
            emit_xtrans0(2)
            emit_warm(4)
            emit_wtransS(wq_stage[4], WqT, 4)
            emit_xtrans0(3)
            emit_wtransS(wq_stage[5], WqT, 5)
            emit_warm(4)
            emit_wtransS(wk_stage[0], WkT, 0)
            emit_wtransS(wk_stage[1], WkT, 1)
            emit_m_group(0)
            emit_wtransS(wk_stage[2], WkT, 2)
            emit_wtransS(wk_stage[3], WkT, 3)
            emit_m_group(1)
            emit_wtransS(wk_stage[4], WkT, 4)
            emit_wtransS(wk_stage[5], WkT, 5)
            emit_m_group(2)

            # ---- V projection for chunk 0 up front: its inputs (XT(0),
            # WV) are ready before Mt, so it fills the tail of the
            # startup window while the last M groups finish.
            def emit_vproj(c):
                for sb in range(QPC):
                    xt_blk = XT[:, :, (c * QPC + sb) * P : (c * QPC + sb + 1) * P]
                    pv0 = psW.tile([P, CH], F32, tag="w")
                    for do in range(ND):
                        nc.tensor.matmul(
                            pv0,
                            xt_blk[:, do, :],
                            WV[:, do, 0:CH],
                            start=(do == 0),
                            stop=(do == ND - 1),
                        )
                    nc.scalar.copy(V[:, c * QPC + sb, 0:CH], pv0)
                    pv1 = psW.tile([P, CH], F32, tag="w")
                    for do in range(ND):
                        nc.tensor.matmul(
                            pv1[:, 0 : D - CH],
                            xt_blk[:, do, :],
                            WV[:, do, CH:D],
                            start=(do == 0),
                            stop=(do == ND - 1),
                        )
                    nc.scalar.copy(V[:, c * QPC + sb, CH:D], pv1[:, 0 : D - CH])

            emit_vproj(0)

            for c in range(NCH):
                # ---- projection: B = (x M)^T = Mt.T-contracted with XT
                QT = qtp.tile([P, ND, CH], BF16, tag="qt")
                for eb in range(ND):
                    pq = psW.tile([P, CH], F32, tag="w")
                    for do in range(ND):
                        nc.tensor.matmul(
                            pq,
                            Mt[:, do, eb * P : (eb + 1) * P],
                            XT[:, do, c * CH : (c + 1) * CH],
                            start=(do == 0),
                            stop=(do == ND - 1),
                        )
                    nc.vector.tensor_copy(QT[:, eb, :], pq)

                # ---- scores^T + exp; triangular mask on the diagonal corner
                for kb in range(QPC * (c + 1)):
                    i = kb - QPC * c
                    q0 = max(i, 0) * P
                    W = CH - q0
                    ps_s = psW.tile([P, CH], F32, tag="w")
                    for eo in range(ND):
                        nc.tensor.matmul(
                            ps_s[:, 0:W],
                            XT[:, eo, kb * P : (kb + 1) * P],
                            QT[:, eo, q0:CH],
                            start=(eo == 0),
                            stop=(eo == ND - 1),
                        )
                    nc.scalar.activation(PT[:, kb, q0:CH], ps_s[:, 0:W], EXP, scale=SCALE)
                    if i >= 0:
                        nc.vector.tensor_mul(
                            PT[:, kb, q0 : q0 + P], PT[:, kb, q0 : q0 + P], tri
                        )

                # ---- V projection for this chunk (chunk 0 ran up front)
                if c > 0:
                    emit_vproj(c)

                # next chunk's XT production rides the sync ring ahead of
                # this chunk's output stores
                if c + 1 < NCH:
                    emit_xt_dma(c + 1)

                # ---- attn @ [V | 1], normalize, store
                for qs in range(QPC):
                    qb = c * QPC + qs
                    po0 = psO.tile([P, CH], F32, tag="o")
                    po1 = psW.tile([P, D + 2 - CH], F32, tag="w")
                    for kb in range(qb + 1):
                        nc.tensor.matmul(
                            po1,
                            PT[:, kb, qs * P : (qs + 1) * P],
                            V[:, kb, CH : D + 2],
                            start=(kb == 0),
                            stop=(kb == qb),
                        )
                    recip = rcp.tile([P, 1], F32, tag="rc")
                    nc.vector.reciprocal(recip, po1[:, D - CH : D - CH + 1])
                    o_sb = outp.tile([P, D], F32, tag="o")
                    nc.scalar.mul(o_sb[:, CH:D], po1[:, 0 : D - CH], recip)
                    nc.sync.dma_start(out_d[qb * P : (qb + 1) * P, CH:D], o_sb[:, CH:D])
                    for kb in range(qb + 1):
                        nc.tensor.matmul(
                            po0,
                            PT[:, kb, qs * P : (qs + 1) * P],
                            V[:, kb, 0:CH],
                            start=(kb == 0),
                            stop=(kb == qb),
                        )
                    nc.scalar.mul(o_sb[:, 0:CH], po0, recip)
                    nc.sync.dma_start(out_d[qb * P : (qb + 1) * P, 0:CH], o_sb[:, 0:CH])

    nc.finalize()
    return nc


_NC_CACHE = None


def _get_nc():
    global _NC_CACHE
    if _NC_CACHE is None:
        _NC_CACHE = _build_nc()
    return _NC_CACHE


def run(inputs, trace=False):
    x = np.asarray(inputs["x"], dtype=np.float32)
    wq = np.asarray(inputs["wq"], dtype=np.float32)
    wk = np.asarray(inputs["wk"], dtype=np.float32)
    wv = np.asarray(inputs["wv"], dtype=np.float32)
    nc = _get_nc()
    in_maps = [
        {"xb": np.ascontiguousarray(x[b]), "wq": wq, "wk": wk, "wv": wv}
        for b in range(B)
    ]
    res = run_bass_kernel_spmd(nc, in_maps, core_ids=list(range(B)), trace=trace)
    out = np.stack([r["out"] for r in res.results]).astype(np.float32)
    return out, res


def kernel(x, wq, wk, wv):
    out, _ = run({"x": x, "wq": wq, "wk": wk, "wv": wv}, trace=False)
    return out
